# revision 23
# baseline (speedup 1.0000x reference)
"""GCN-Cat message-passing kernel for 8 trn2 NeuronCores.

Strategy:
  - GCNConv is linear before relu: aggregate input features over edges, then
    apply W. With the concat structure, each layer only aggregates the newly
    produced features (8 / 64 / 128 dims instead of 64 / 128 / 256).
  - Nodes relabeled so graphs are contiguous + padded to 128-multiples, whole
    graphs assigned to cores -> per-graph max pool becomes per-128-window max.
  - Edges sharded by dst core/block; segment-sum via one-hot matmuls on PE
    (PSUM accumulate); per-edge gathers via gpsimd.dma_gather (256B rows).
  - Layer-1 messages are host-expanded (inp is a kernel input -> pure data
    layout), so layer 1 needs no device gather at all.
  - Single-precision tables: h1 as fp32 (gather rows must be 256B multiples),
    h2 as fp16.  Relu emits the table dtype directly on the scalar engine.
  - Gathers are merged into multi-block groups to amortize the ~1us fixed
    swdge overhead; the table split point matches the AllGather stage
    boundary so half-0 gathers only wait on AG stage 0.
"""
import contextlib
import sys

import numpy as np

sys.path.insert(0, '/opt/trn_rl_repo')

import concourse.bacc as bacc
import concourse.mybir as mybir
import concourse.tile as tile
from concourse.library_config import mlp

NCORES = 8
P = 128
MAXPC = 20   # target chunks per gather (~2200-2500 descs is the swdge sweet spot)


def _ceil(a, b):
    return int(-(-a // b))


class Meta:
    pass


def preprocess(inputs, G=32):
    """Host-side prep: relabel nodes, shard/sort/pad edges, build per-core arrays."""
    norm, pos, x = (np.asarray(inputs[k]) for k in ('norm', 'pos', 'x'))
    edge_index = np.asarray(inputs['edge_index'])
    batch = np.asarray(inputs['batch']).astype(np.int64)
    N = norm.shape[0]

    inp = np.concatenate([norm, pos, x], axis=1).astype(np.float32)  # [N, 8]

    counts = np.bincount(batch, minlength=G)
    starts = np.concatenate([[0], np.cumsum(counts)])
    gblocks = [_ceil(int(c), P) for c in counts]

    # assign graphs to cores, balancing padded block counts (LPT)
    core_blocks = [0] * NCORES
    core_graphs = [[] for _ in range(NCORES)]
    for g in sorted(range(G), key=lambda g: -gblocks[g]):
        k = int(np.argmin(core_blocks))
        core_blocks[k] += gblocks[g]
        core_graphs[k].append(g)
    B = max(max(core_blocks), 1)  # blocks per core (uniform)
    NLOC = B * P
    NFULL = NCORES * NLOC

    # node permutation + per-graph window map
    perm = np.zeros(N, np.int64)
    gwin = {}
    for k in range(NCORES):
        off = k * NLOC
        w = 0
        for g in core_graphs[k]:
            n = int(counts[g])
            if n == 0:
                continue
            perm[starts[g]:starts[g + 1]] = off + w * P + np.arange(n)
            gwin[g] = (k, w, w + _ceil(n, P))
            w += _ceil(n, P)

    src = perm[edge_index[0].astype(np.int64)]
    dst = perm[edge_index[1].astype(np.int64)]

    # stage-major table index: tables laid out [stage0: 8 x HL0 | stage1: 8 x HL1]
    B0 = _ceil(B, 2)            # stage-0 blocks per core
    HL0, HL1 = B0 * P, (B - B0) * P
    split = NCORES * HL0        # == AG stage boundary; both halves < 32768
    assert split <= 32768 and NFULL - split <= 32768
    kk = np.arange(NFULL) // NLOC
    rr = np.arange(NFULL) % NLOC
    tidx_map = np.where(rr < HL0,
                        kk * HL0 + rr,
                        NCORES * HL0 + kk * HL1 + (rr - HL0)).astype(np.int64)
    tsrc = tidx_map[src]

    blk = dst // P
    half = (tsrc >= split).astype(np.int64)
    order = np.lexsort((tsrc, half, blk))
    src_s, dst_s = src[order], dst[order]
    tsrc_s = tsrc[order]
    key_s = blk[order] * 2 + half[order]

    cnt = np.bincount(key_s, minlength=NCORES * B * 2).reshape(NCORES, B, 2)
    caps = np.zeros((B, 2), np.int64)
    caps[:, 0] = [_ceil(int(v), P) for v in cnt[:, :, 0].max(axis=0)]
    caps[:, 1] = [_ceil(int(v), P) for v in cnt[:, :, 1].max(axis=0)]
    for b in range(B):
        if caps[b].sum() == 0:
            caps[b, 0] = 1

    # merged gather groups: consecutive blocks, one gather per (group, half).
    # pieces: (half, [(block, cap), ...], cg0) with sum(caps) <= MAXPC
    groups = []          # list of lists of block ids
    cur, tot = [], 0
    for b in range(B):
        c = int(caps[b].sum())
        if cur and tot + c > 2 * MAXPC:
            groups.append(cur)
            cur, tot = [], 0
        cur.append(b)
        tot += c
    if cur:
        groups.append(cur)

    # staggered piece order: group g's half-1 piece comes STAGGER pieces
    # after its half-0 piece, so the gather stream has stage-0-only work to
    # chew on while each layer's stage-1 AllGather completes.
    STAGGER = 5
    raw = {0: [], 1: []}
    for gid, grp in enumerate(groups):
        for h in range(2):
            blks = [(b, int(caps[b, h])) for b in grp]
            pc = sum(c for _, c in blks)
            if pc == 0:
                continue
            raw[h].append((h, blks, pc, gid))
    order = []
    i0 = i1 = 0
    while i0 < len(raw[0]) or i1 < len(raw[1]):
        if i0 < len(raw[0]):
            order.append(raw[0][i0])
            i0 += 1
        if i0 >= STAGGER or i0 >= len(raw[0]):
            if i1 < len(raw[1]):
                order.append(raw[1][i1])
                i1 += 1
    pieces = []   # (half, [(b, cap_bh), ...], cg0, pc, gid)
    coff = 0
    for h, blks, pc, gid in order:
        pieces.append((h, blks, coff, pc, gid))
        coff += pc
    tot_chunks = coff
    maxpc = max(p[3] for p in pieces)
    n_groups = len(groups)

    eoff = np.concatenate([[0], np.cumsum(np.bincount(
        key_s, minlength=NCORES * B * 2))]).astype(np.int64)

    inp_new = np.zeros((NFULL, 8), np.float32)
    inp_new[perm] = inp

    # chunk start per (b, h) in the merged layout
    chunk_of = {}
    cg = 0
    for h, blks, cg0, pc, gid in pieces:
        c = cg0
        for b, cap in blks:
            chunk_of[(b, h)] = (c, cap)
            c += cap

    cores = []
    for k in range(NCORES):
        slot_src = np.zeros(tot_chunks * P, np.int64)
        slot_tsrc = np.zeros(tot_chunks * P, np.int64)
        dst_vals = np.full(tot_chunks * P, 255.0, np.float32)
        for (b, h), (c0, cap) in chunk_of.items():
            key = (k * B + b) * 2 + h
            s0, s1 = int(eoff[key]), int(eoff[key + 1])
            n_here = s1 - s0
            if n_here > 0:
                sl = slice(c0 * P, c0 * P + n_here)
                slot_src[sl] = src_s[s0:s1]
                slot_tsrc[sl] = tsrc_s[s0:s1]
                dst_vals[sl] = (dst_s[s0:s1] % P).astype(np.float32)
        idx_parts = []
        for h, blks, cg0, pc, gid in pieces:
            ids = slot_tsrc[cg0 * P:(cg0 + pc) * P].copy()
            if h == 1:
                ids = ids - split
                ids[ids < 0] = 0
            lay = ids.astype(np.int32).reshape(pc * 8, 16).T.astype(np.int16)
            idx_parts.append(np.tile(lay, (8, 1)))
        m1 = inp_new[slot_src].astype(BF16)       # [slots, 8]
        cores.append(dict(
            idx=np.ascontiguousarray(np.concatenate(idx_parts, axis=1)),
            dstb=np.ascontiguousarray(dst_vals.reshape(tot_chunks, P).T.astype(BF16)),
            msg1=np.ascontiguousarray(
                m1.reshape(tot_chunks, P, 8).transpose(1, 0, 2).reshape(P, tot_chunks * 8)),
        ))

    W1, b1 = np.asarray(inputs['W1'], np.float32), np.asarray(inputs['b1'], np.float32)
    W2, b2 = np.asarray(inputs['W2'], np.float32), np.asarray(inputs['b2'], np.float32)
    W3, b3 = np.asarray(inputs['W3'], np.float32), np.asarray(inputs['b3'], np.float32)
    Wl, bl = np.asarray(inputs['Wl'], np.float32), np.asarray(inputs['bl'], np.float32)
    F1, F2, F3, C = W1.shape[1], W2.shape[1], W3.shape[1], Wl.shape[1]
    # stack row layout: stack1 = [A1(F1) | A0(8) | ones] ; stack2 = [A2(F2)]
    w1eff = np.concatenate([W1, b1[None, :]], 0)                       # [9, F1]
    w2eff = np.concatenate([W2[:F1], W2[F1:F1 + 8], b2[None, :]], 0)   # [F1+9, F2]
    w3a = np.concatenate([W3[:F1], W3[F1:F1 + 8] + W3[F1 + 8 + F2:],
                          b3[None, :]], 0)                             # [F1+9, F3]
    w3b = W3[F1 + 8:F1 + 8 + F2]                                       # [F2, F3]

    m = Meta()
    m.G, m.C, m.split = G, C, split
    m.B, m.NLOC, m.NFULL = B, NLOC, NFULL
    m.F1, m.F2, m.F3 = F1, F2, F3
    m.pieces, m.tot_chunks, m.maxpc = pieces, tot_chunks, maxpc
    m.caps = caps
    m.n_groups = n_groups
    m.gwin, m.perm = gwin, perm
    m.B0, m.HL0, m.HL1 = B0, HL0, HL1
    m.weights = dict(w1eff=w1eff, w2eff=w2eff, w3a=w3a, w3b=w3b, wl=Wl,
                     bl=bl[None, :].astype(np.float32))
    m.cores = cores
    return m


def build(m):
    """Build the SPMD Tile program (identical across cores)."""
    fp32, bf16, i16 = mybir.dt.float32, mybir.dt.bfloat16, mybir.dt.int16
    F1, F2, F3, B, G, C = m.F1, m.F2, m.F3, m.B, m.G, m.C
    NLOC, NFULL, TC = m.NLOC, m.NFULL, m.tot_chunks
    KA = F1 + 9        # stack1 active rows (A1, A0, ones)
    FH = F3 // 2
    AF = mybir.ActivationFunctionType

    nc = bacc.Bacc("TRN2", target_bir_lowering=False, debug=False,
                   num_devices=NCORES, num_swdge_queues=4)

    p_msg1 = nc.dram_tensor("msg1", [P, TC * 8], bf16, kind="ExternalInput")
    p_idx = nc.dram_tensor("idxb", [P, TC * 8], i16, kind="ExternalInput")
    p_dstb = nc.dram_tensor("dstb", [P, TC], bf16, kind="ExternalInput")
    p_iotb = nc.dram_tensor("iotb", [P, m.maxpc * P], bf16, kind="ExternalInput")
    p_w1 = nc.dram_tensor("w1eff", [9, F1], fp32, kind="ExternalInput")
    p_w2 = nc.dram_tensor("w2eff", [KA, F2], fp32, kind="ExternalInput")
    p_w3a = nc.dram_tensor("w3a", [KA, F3], fp32, kind="ExternalInput")
    p_w3b = nc.dram_tensor("w3b", [F2, F3], fp32, kind="ExternalInput")
    p_wl = nc.dram_tensor("wl", [F3, C], fp32, kind="ExternalInput")
    p_bl = nc.dram_tensor("bl", [1, C], fp32, kind="ExternalInput")
    o_out = nc.dram_tensor("o_out", [G, C], fp32, kind="ExternalOutput")
    o_pred = nc.dram_tensor("o_pred", [G, C], fp32, kind="ExternalOutput")

    h1_loc = nc.dram_tensor("h1_loc", [NLOC, F1], fp32)
    h2_loc = nc.dram_tensor("h2_loc", [NLOC, F2], bf16)
    wm_loc = nc.dram_tensor("wm_loc", [2, P, B], fp32)
    h1_full = nc.dram_tensor("h1_full", [NFULL, F1], fp32, addr_space="Shared")
    h2_full = nc.dram_tensor("h2_full", [NFULL, F2], bf16, addr_space="Shared")
    wm_full = nc.dram_tensor("wm_full", [NCORES * 2, P, B], fp32, addr_space="Shared")

    rg = [list(range(NCORES))]

    # balanced queue assignment per gather piece (greedy by chunk count)
    qload = [0] * 4
    piece_q = []
    for h, blks, cg0, pc, gid in m.pieces:
        q = int(np.argmin(qload))
        qload[q] += pc
        piece_q.append(q)

    with tile.TileContext(nc) as tc:
        nc.gpsimd.load_library(mlp)
        with contextlib.ExitStack() as ctx:
            const = ctx.enter_context(tc.tile_pool(name="const", bufs=1))
            ohp = ctx.enter_context(tc.tile_pool(name="oh", bufs=4))
            msgp = ctx.enter_context(tc.tile_pool(name="msg", bufs=4))
            hstp = ctx.enter_context(tc.tile_pool(name="hst", bufs=4))
            accp = ctx.enter_context(tc.tile_pool(name="acc", bufs=5, space="PSUM"))
            epip = ctx.enter_context(tc.tile_pool(name="epi", bufs=1, space="PSUM"))
            finp = ctx.enter_context(tc.tile_pool(name="fin", bufs=2, space="PSUM"))

            idx_sb = const.tile([P, TC * 8], i16)
            dstb_sb = const.tile([P, TC], bf16)
            iotb_sb = const.tile([P, m.maxpc * P], bf16)
            stack1 = const.tile([P, B * P], fp32)
            stack2 = const.tile([P, B * P], fp32)
            w1_sb = const.tile([P, F1], fp32)  # rows F1:F1+9 hold w1eff (base-64 match)
            w2_sb = const.tile([KA, F2], fp32)
            w3a_sb = [const.tile([KA, FH], fp32, tag=f"w3a{fh}", name=f"w3a{fh}") for fh in range(2)]
            w3b_sb = [const.tile([F2, FH], fp32, tag=f"w3b{fh}", name=f"w3b{fh}") for fh in range(2)]
            wl_sb = [const.tile([FH, C], fp32, tag=f"wl{fh}", name=f"wl{fh}") for fh in range(2)]
            bl_sb = const.tile([1, C], fp32)
            wmax = [const.tile([P, B], fp32, tag=f"wmax{fh}", name=f"wmax{fh}") for fh in range(2)]
            pooled = [const.tile([P, G], fp32, tag=f"pool{fh}", name=f"pool{fh}") for fh in range(2)]
            pw_sb = const.tile([P, NCORES * 2 * B], fp32)
            soft = const.tile([G, 6 * C + 8], fp32)
            ones_g = const.tile([1, G], fp32)

            nc.sync.dma_start(idx_sb[:], p_idx[:])
            nc.sync.dma_start(dstb_sb[:], p_dstb[:])
            nc.sync.dma_start(iotb_sb[:], p_iotb[:])
            nc.sync.dma_start(w1_sb[F1:F1 + 9, :], p_w1[:])
            nc.sync.dma_start(w2_sb[:], p_w2[:])
            for fh in range(2):
                fsl = slice(fh * FH, (fh + 1) * FH)
                nc.sync.dma_start(w3a_sb[fh][:], p_w3a[:, fsl])
                nc.sync.dma_start(w3b_sb[fh][:], p_w3b[:, fsl])
                nc.sync.dma_start(wl_sb[fh][:], p_wl[fsl, :])
            nc.sync.dma_start(bl_sb[:], p_bl[:])
            nc.vector.memset(stack1[F1:F1 + 32, :], 1.0)  # ones row at F1+8; A0 copies overwrite F1:F1+8
            nc.vector.memset(ones_g[:], 1.0)

            def final_epilogue(layer, b):
                cols = slice(b * P, (b + 1) * P)
                if layer == 1:
                    h = epip.tile([P, F2], fp32, tag="epi", name="epi_t")
                    nc.tensor.matmul(h[:, :F1], stack1[F1:F1 + 9, cols],
                                     w1_sb[F1:F1 + 9, :], start=True, stop=True)
                    hf = hstp.tile([P, F1], fp32, tag="hst1", name="hst1_t")
                    nc.scalar.activation(hf[:], h[:, :F1], AF.Relu)
                    nc.sync.dma_start(h1_loc[b * P:(b + 1) * P, :], hf[:])
                elif layer == 2:
                    h = epip.tile([P, F2], fp32, tag="epi", name="epi_t")
                    nc.tensor.matmul(h[:], stack1[0:KA, cols], w2_sb[:],
                                     start=True, stop=True)
                    hb = hstp.tile([P, F2], f16, tag="hst2", name="hst2_t")
                    nc.scalar.activation(hb[:], h[:], AF.Relu)
                    nc.sync.dma_start(h2_loc[b * P:(b + 1) * P, :], hb[:])
                else:
                    for fh in range(2):
                        h3 = finp.tile([P, P], fp32, tag="fin", name="fin_t")
                        nc.tensor.matmul(h3[:], w3a_sb[fh][:], stack1[0:KA, cols],
                                         start=True, stop=False)
                        nc.tensor.matmul(h3[:], w3b_sb[fh][:], stack2[:, cols],
                                         start=False, stop=True)
                        hr = hstp.tile([P, P], fp32, tag="hst3", name="hst3_t")
                        nc.scalar.activation(hr[:], h3[:], AF.Relu)
                        nc.vector.reduce_max(out=wmax[fh][:, b:b + 1], in_=hr[:],
                                             axis=mybir.AxisListType.X)

            def stack_ap(layer, b):
                cols = slice(b * P, (b + 1) * P)
                if layer == 1:
                    return stack1[F1:F1 + 8, cols]
                if layer == 2:
                    return stack1[0:F1, cols]
                return stack2[:, cols]

            def acc_ap(layer, acc):
                if layer == 1:
                    return acc[F1:F1 + 8, :]
                if layer == 2:
                    return acc[0:F1, :]
                return acc[:, :]

            def emit_piece(layer, pi, accs, started):
                h, blks, cg0, pc, gid = m.pieces[pi]
                F = 8 if layer == 1 else (F1 if layer == 2 else F2)
                if layer == 1:
                    m1t = msgp.tile([P, m.maxpc * 8], f16, tag="msg1",
                                    bufs=4, name="m1_t")
                    nc.sync.dma_start(m1t[:, :pc * 8],
                                      p_msg1[:, cg0 * 8:(cg0 + pc) * 8])
                    st_all = m1t
                elif layer == 2:
                    msgf = msgp.tile([P, m.maxpc, F1], fp32, tag="msg2f",
                                     bufs=4, name="msgf_t")
                    src_ap = (h1_full[0:m.split, :] if h == 0
                              else h1_full[m.split:NFULL, :])
                    nc.gpsimd.dma_gather(
                        msgf[:, :pc, :], src_ap,
                        idx_sb[:, cg0 * 8:(cg0 + pc) * 8],
                        pc * P, pc * P, F1,
                        queue_num=piece_q[pi], single_packet=False)
                    msgb = msgp.tile([P, m.maxpc, F1], f16, tag="msg2b",
                                     bufs=4, name="msgb_t")
                    nc.scalar.copy(msgb[:, :pc, :], msgf[:, :pc, :])
                    st_all = msgb
                else:
                    msg = msgp.tile([P, m.maxpc, F2], f16, tag="msg3",
                                    bufs=4, name="msg_t")
                    src_ap = (h2_full[0:m.split, :] if h == 0
                              else h2_full[m.split:NFULL, :])
                    nc.gpsimd.dma_gather(
                        msg[:, :pc, :], src_ap,
                        idx_sb[:, cg0 * 8:(cg0 + pc) * 8],
                        pc * P, pc * P, F2,
                        queue_num=piece_q[pi], single_packet=False)
                    st_all = msg
                oh = ohp.tile([P, m.maxpc, P], f16, tag="oh", name="oh_t")
                nc.vector.tensor_tensor(
                    out=oh[:, :pc, :],
                    in0=dstb_sb[:, cg0:cg0 + pc, None].to_broadcast([P, pc, P]),
                    in1=iotb_sb[:, :pc * P].rearrange("p (c q) -> p c q", q=P),
                    op=mybir.AluOpType.is_equal)
                c = 0
                for b, cap in blks:
                    if cap == 0:
                        continue
                    acc = accp.tile([P, P], fp32, tag="acc", name="acc_t")
                    for cc in range(cap):
                        if layer == 1:
                            st = st_all[:, (c + cc) * 8:(c + cc) * 8 + 8]
                            w, orow, tpos = 8, F1, (0, F1)
                        else:
                            st = st_all[:, c + cc, :]
                            w, orow, tpos = F, 0, None
                        nc.tensor.matmul(
                            acc[orow:orow + w, :], st, oh[:, c + cc, :],
                            start=(cc == 0), stop=(cc == cap - 1),
                            tile_position=tpos)
                    c += cap
                    if h == 0:
                        nc.scalar.copy(stack_ap(layer, b), acc_ap(layer, acc))
                        if int(m.caps[b, 1]) == 0:
                            final_epilogue(layer, b)
                    else:
                        if int(m.caps[b, 0]) == 0:
                            nc.scalar.copy(stack_ap(layer, b), acc_ap(layer, acc))
                        else:
                            nc.vector.tensor_tensor(
                                out=stack_ap(layer, b),
                                in0=acc_ap(layer, acc),
                                in1=stack_ap(layer, b),
                                op=mybir.AluOpType.add)
                        final_epilogue(layer, b)

            HL0, HL1 = m.HL0, m.HL1
            n_p = len(m.pieces)

            def ag_stage(loc, full, stage):
                if stage == 0:
                    nc.gpsimd.collective_compute(
                        "AllGather", mybir.AluOpType.bypass, replica_groups=rg,
                        ins=[loc[0:HL0, :].opt()],
                        outs=[full[0:NCORES * HL0, :].opt()])
                else:
                    nc.gpsimd.collective_compute(
                        "AllGather", mybir.AluOpType.bypass, replica_groups=rg,
                        ins=[loc[HL0:NLOC, :].opt()],
                        outs=[full[NCORES * HL0:NFULL, :].opt()])

            for layer in (1, 2, 3):
                accs, started = {}, {}
                for pi in range(n_p):
                    emit_piece(layer, pi, accs, started)
                if layer == 1:
                    ag_stage(h1_loc, h1_full, 0)
                    ag_stage(h1_loc, h1_full, 1)
                elif layer == 2:
                    ag_stage(h2_loc, h2_full, 0)
                    ag_stage(h2_loc, h2_full, 1)

            for fh in range(2):
                nc.sync.dma_start(wm_loc[fh, :, :], wmax[fh][:])
            nc.gpsimd.collective_compute(
                "AllGather", mybir.AluOpType.bypass, replica_groups=rg,
                ins=[wm_loc.ap().opt()], outs=[wm_full.ap().opt()])
            nc.sync.dma_start(
                pw_sb[:].rearrange("p (i b) -> p i b", b=B),
                wm_full.ap().rearrange("i p b -> p i b"))
            for g in range(G):
                if g in m.gwin:
                    k, w0, w1 = m.gwin[g]
                    for fh in range(2):
                        i = k * 2 + fh
                        nc.vector.reduce_max(
                            out=pooled[fh][:, g:g + 1],
                            in_=pw_sb[:, i * B + w0:i * B + w1],
                            axis=mybir.AxisListType.X)
                else:
                    for fh in range(2):
                        nc.vector.memset(pooled[fh][:, g:g + 1], 0.0)

            lg = epip.tile([P, C], fp32, tag="epi", name="lg_t")
            nc.tensor.matmul(lg[:G, :], pooled[0][:], wl_sb[0][:],
                             start=True, stop=False)
            nc.tensor.matmul(lg[:G, :], pooled[1][:], wl_sb[1][:],
                             start=False, stop=False)
            nc.tensor.matmul(lg[:G, :], ones_g[:], bl_sb[:],
                             start=False, stop=True)

            z, zs = soft[:, 0:C], soft[:, C:2 * C]
            e, ot = soft[:, 2 * C:3 * C], soft[:, 3 * C:4 * C]
            pr = soft[:, 4 * C:5 * C]
            mx, sm = soft[:, 5 * C:5 * C + 1], soft[:, 5 * C + 1:5 * C + 2]
            ls, ri = soft[:, 5 * C + 2:5 * C + 3], soft[:, 5 * C + 3:5 * C + 4]
            nc.vector.tensor_copy(out=z, in_=lg[:G, :])
            nc.vector.reduce_max(out=mx, in_=z, axis=mybir.AxisListType.X)
            nc.vector.tensor_scalar(out=zs, in0=z, scalar1=mx, scalar2=None,
                                    op0=mybir.AluOpType.subtract)
            nc.scalar.activation(e, zs, AF.Exp)
            nc.vector.reduce_sum(out=sm, in_=e, axis=mybir.AxisListType.X)
            nc.scalar.activation(ls, sm, AF.Ln)
            nc.vector.reciprocal(ri, sm)
            nc.vector.tensor_scalar(out=ot, in0=zs, scalar1=ls, scalar2=None,
                                    op0=mybir.AluOpType.subtract)
            nc.vector.tensor_scalar(out=pr, in0=e, scalar1=ri, scalar2=None,
                                    op0=mybir.AluOpType.mult)
            nc.sync.dma_start(o_out[:], ot)
            nc.sync.dma_start(o_pred[:], pr)

    nc.compile()
    return nc


def make_in_maps(m):
    iota = np.ascontiguousarray(
        np.tile(np.arange(P, dtype=np.float32), m.maxpc)[None, :].repeat(P, 0))
    w = m.weights
    shared = {"iotb": iota.astype(BF16),
              "w1eff": w['w1eff'], "w2eff": w['w2eff'], "w3a": w['w3a'],
              "w3b": w['w3b'], "wl": w['wl'], "bl": w['bl']}
    return [{**shared, "msg1": c['msg1'], "idxb": c['idx'],
             "dstb": c['dstb']} for c in m.cores]


def run(inputs, G=32, trace=False):
    from concourse.bass_utils import run_bass_kernel_spmd
    m = preprocess(inputs, G=G)
    nc = build(m)
    maps = make_in_maps(m)
    res = run_bass_kernel_spmd(nc, maps, list(range(NCORES)), trace=trace)
    out = np.asarray(res.results[0]["o_out"])
    pred = np.asarray(res.results[0]["o_pred"])
    return (out, pred), res


def kernel(**inputs):
    """Full-inputs -> full-output GCN forward on 8 trn2 NeuronCores."""
    from concourse.bass_utils import run_bass_kernel_spmd
    m = preprocess(inputs, G=32)
    nc = build(m)
    maps = make_in_maps(m)
    res = run_bass_kernel_spmd(nc, maps, list(range(NCORES)), trace=False)
    out = np.asarray(res.results[0]["o_out"], dtype=np.float32)
    pred = np.asarray(res.results[0]["o_pred"], dtype=np.float32)
    return (out, pred)
